# revision 70
# baseline (speedup 1.0000x reference)
"""Trainium2 Bass kernel v9 for nn_EnhancedQuantumLayer (10-qubit, 4-layer
variational circuit, batch 512, Z-expectations output).

Data parallel over 8 cores, 64 samples/core. Changes vs the v2 baseline
(58.3us -> ~47us):
  - PE warm-up chain (~4.3us of dep-free dummy matmuls) releases the HAM
    clock gate before the real matmuls; data-pinned heartbeat matmuls
    refill later PE-idle stretches (HAM re-throttles on idle windows).
  - Act-table steering Silu reads a memset tile (no DMA dependency) so
    it schedules first on ACT and one table set serves tanh/sin/sq/copy.
  - Consts consolidated: one f32 pack (3 pipelined DMAs) + the bf16
    weight stack; the 160KB dmask is gone - the cos bias rides a
    ones-row in the spread so one matmul per chunk gives
    omega*t/2pi + bias-in-turns.
  - Range reduction: fused magic-round tensor_scalar (+M, -M) + one STT;
    sin(2pi*diff) via the activation scale field.
  - Doubling chain fused into a 3-level planar (re/im) kron tree,
    ~19 DVE ops instead of ~35, via the QORD column layout.
  - Batch bit b0 kept innermost in both layouts so the layer-loop
    32x32 block transposes run on uint32 pairs (444ns vs 712ns); the
    B-space op embeds identity on b1 instead of b0 (same matrix).
  - Measurement: dual-permutation squares feed plain-moving W1 matmuls;
    one stream transpose + stationary-data matmul gives q0..4 in
    sample partitions, another + strided reduce gives q5..9; output is
    3 clean DMAs (rows come back (b>>2, b0, b1)-ordered; host applies
    ROWPERM when gathering).
  - GPSIMD touched mid-kernel and near the tail so the postamble
    semaphore range-clear pays no wake-up.

Host precompute is theta-only: 8 realified 128x128 stationaries (bf16).
"""

import math

import ml_dtypes
import numpy as np

N_QUBITS = 10
N_LAYERS = 4
FREQS = (1.0, 2.0, 4.0, 8.0, 16.0)
PI = float(np.pi)
B_TOTAL = 512
B_CORE = 64
N_CORES = 8
WARM_MM = 26
FUSED_ROUND = True
TR_FROM_PSUM = False

CZCNOT = np.array([[1, 0, 0, 0],
                   [0, 1, 0, 0],
                   [0, 0, 0, -1],
                   [0, 0, 1, 0]], dtype=np.complex128)

# vT column order: L1 kron pairs are g = (QORD[g], QORD[5+g]), laid out
# as (5,0) (1,2) (6,7) (8,9) (3,4) so the L2 operands p50/p67 (X) and
# p12/p89 (Y) sit at regular 16-col strides and p34 is contiguous.
QORD = (5, 1, 6, 8, 3, 0, 2, 7, 9, 4)

# device output rows come back as r = (b>>2, b0, b1); ROWPERM[r] = b
ROWPERM = np.array([(r & ~3) + 2 * (r & 1) + ((r >> 1) & 1)
                    for r in range(B_CORE)])


# ---------------------------------------------------------------- host math
def _rz(phi):
    return np.array([[np.exp(-0.5j * phi), 0], [0, np.exp(0.5j * phi)]])


def _rx(th):
    c, s = np.cos(th / 2), np.sin(th / 2)
    return np.array([[c, -1j * s], [-1j * s, c]])


def _ry(th):
    c, s = np.cos(th / 2), np.sin(th / 2)
    return np.array([[c, -s], [s, c]])


def _kron_list(ms):
    out = ms[0]
    for m in ms[1:]:
        out = np.kron(out, m)
    return out


def _embed_2q(space_qubits, qa, qb, M4):
    n = len(space_qubits)
    dim = 2 ** n
    pa, pb = space_qubits.index(qa), space_qubits.index(qb)
    out = np.zeros((dim, dim), dtype=np.complex128)
    for idx in range(dim):
        bits = [(idx >> (n - 1 - i)) & 1 for i in range(n)]
        col4 = 2 * bits[pa] + bits[pb]
        for row4 in range(4):
            val = M4[row4, col4]
            if val != 0:
                nb = bits.copy()
                nb[pa], nb[pb] = row4 >> 1, row4 & 1
                ridx = sum(bit << (n - 1 - i) for i, bit in enumerate(nb))
                out[ridx, idx] += val
    return out


def _realify(M):
    return np.block([[M.real, -M.imag], [M.imag, M.real]])


def _embed_OB(M_L):
    """layout-B partition op on (q5, b0, q6..q9): identity on b0."""
    M = M_L.reshape(2, 16, 2, 16)
    O = np.zeros((2, 2, 16, 2, 2, 16), np.complex128)
    for b0 in range(2):
        O[:, b0, :, :, b0, :] = M
    return O.reshape(64, 64)


def _host_weights(theta):
    """wstack [128, 8, 128] bf16: per layer [lhsT_A, lhsT_L] with
    lhsT = realify(op).T, partition-major for one contiguous DMA."""
    ang = np.tanh(theta.astype(np.float64)) * PI
    A_space = [5, 0, 1, 2, 3, 4]
    L_space = [5, 6, 7, 8, 9]
    mats = []
    for l in range(N_LAYERS):
        U = []
        for q in range(10):
            a0, a1, a2 = ang[l, q]
            U.append(_rx(a0 * 0.5) @ _rz(a2) @ _ry(a1) @ _rz(a0))
        UA = _kron_list([U[q] for q in A_space])
        E_even_A = (_embed_2q(A_space, 0, 1, CZCNOT)
                    @ _embed_2q(A_space, 2, 3, CZCNOT)
                    @ _embed_2q(A_space, 4, 5, CZCNOT))
        E_odd_A = (_embed_2q(A_space, 1, 2, CZCNOT)
                   @ _embed_2q(A_space, 3, 4, CZCNOT))
        M_A = E_odd_A @ E_even_A @ UA
        UL = _kron_list([np.eye(2)] + [U[q] for q in [6, 7, 8, 9]])
        E_even_L = (_embed_2q(L_space, 6, 7, CZCNOT)
                    @ _embed_2q(L_space, 8, 9, CZCNOT))
        E_odd_L = (_embed_2q(L_space, 5, 6, CZCNOT)
                   @ _embed_2q(L_space, 7, 8, CZCNOT))
        M_L = E_odd_L @ E_even_L @ UL
        mats.append(_realify(M_A).T)
        mats.append(_realify(_embed_OB(M_L)).T)
    stk = np.stack(mats)  # [8, 128, 128]
    return np.ascontiguousarray(stk.transpose(1, 0, 2))  # [128, 8, 128] f64


# ------------------------------------------------------- fourier basis (v)
def _v_of_t(t):
    t = np.atleast_1d(np.asarray(t, np.float64))
    v = np.zeros((t.size, 2), np.complex128)
    v[:, 0] = 1.0
    for f in FREQS:
        phi = f * t
        v = v * np.stack([np.exp(-0.5j * phi), np.exp(0.5j * phi)], -1)
        th = 0.25 * f * t
        c, s = np.cos(th), np.sin(th)
        v = np.stack([c * v[:, 0] - 1j * s * v[:, 1],
                      -1j * s * v[:, 0] + c * v[:, 1]], -1)
    return v


def _fourier_C():
    """C [94, 4]: rows 0-46 sin(0.25 m t), rows 47-93 cos, m = 1,3..93;
    comps (ar, ai, br, bi)."""
    N = 1024
    ts = np.arange(N) * (8 * np.pi / N)
    vv = _v_of_t(ts)
    comps = np.stack([vv[:, 0].real, vv[:, 0].imag,
                      vv[:, 1].real, vv[:, 1].imag], -1)
    F = np.fft.rfft(comps, axis=0)
    msk = np.arange(1, 94, 2)
    a_cos = 2.0 * F[msk].real / N
    b_sin = -2.0 * F[msk].imag / N
    return msk, np.concatenate([b_sin, a_cos], 0).astype(np.float64)


# ------------------------------------------------------------- const packs
def _w1_matrix():
    # W1 [128, 32]: col = b0*16 + o; o: 0 = plain sum, 1..5 = s(q5, q6..9)
    W1 = np.zeros((128, 32), np.float32)
    for p in range(128):
        q5 = (p >> 5) & 1
        b0 = (p >> 4) & 1
        j4 = p & 15
        s = [1 - 2 * q5] + [1 - 2 * ((j4 >> (3 - k)) & 1) for k in range(4)]
        W1[p, b0 * 16 + 0] = 1.0
        for k in range(5):
            W1[p, b0 * 16 + 1 + k] = s[k]
    return W1


def _ca_matrix():
    # Ca [32, 5]: sign of bit q (q0 = MSB of j5) for q = 0..4
    Ca = np.zeros((32, 5), np.float32)
    for j in range(32):
        for q in range(5):
            Ca[j, q] = 1 - 2 * ((j >> (4 - q)) & 1)
    return Ca


NF_F32 = 280


def _const_f32():
    msk, C94 = _fourier_C()
    cf = np.zeros((128, NF_F32), np.float32)
    cf[0:94, 2:6] = C94
    cf[0:4, 6:10] = np.eye(4)
    cf[0:64, 10] = -1.0
    cf[0:64, 11] = 1.0
    cf[0:64, 16:80] = np.eye(64)
    # omega/bias stationary [65, 128]: rows 0..63 = om/2pi, row 64 = bias/2pi
    om = np.zeros(128, np.float64)
    om[0:47] = 0.25 * msk
    om[47:94] = 0.25 * msk
    cf[0:64, 80:208] = (om / (2 * np.pi)).astype(np.float32)
    bias = np.zeros(128, np.float32)
    bias[47:94] = 0.25  # pi/2 in turns
    cf[64, 80:208] = bias
    # umask2 [64, 32]: col = b0*16 + u4; delta(u4 = (b%32)>>1, b0 = b&1)
    for b in range(64):
        cf[b, 208 + (b & 1) * 16 + ((b % 32) >> 1)] = 1.0
    cf[0:32, 240:245] = _ca_matrix()
    # W1 [128, 32] bf16 bit-packed into f32 cols 248:264
    w1b = _w1_matrix().astype(ml_dtypes.bfloat16).view(np.uint16)
    cfu = cf.view(np.uint32)
    cfu[:, 248:264] = w1b[:, 0::2].astype(np.uint32) | (
        w1b[:, 1::2].astype(np.uint32) << 16)
    return cf


NB_W = 1024


def _pack_w(theta):
    wstack = _host_weights(theta)  # [128, 8, 128] f64
    return np.ascontiguousarray(
        wstack.reshape(128, NB_W).astype(ml_dtypes.bfloat16))


# ------------------------------------------------------------- bass builder
_BUILD_CACHE = {}


def _build_module():
    import concourse.bass as bass
    import concourse.mybir as mybir
    from concourse import bacc
    from concourse.tile import TileContext

    f32 = mybir.dt.float32
    f32r = mybir.dt.float32r
    bf16 = mybir.dt.bfloat16
    u32 = mybir.dt.uint32
    AF = mybir.ActivationFunctionType
    OP = mybir.AluOpType

    nc = bacc.Bacc("TRN2", target_bir_lowering=False, debug=False)

    xin = nc.dram_tensor("xin", [B_CORE, 10], f32, kind="ExternalInput").ap()
    wpack = nc.dram_tensor("wpack", [128, NB_W], bf16,
                           kind="ExternalInput").ap()
    out_d = nc.dram_tensor("out", [B_CORE, 10], f32, kind="ExternalOutput").ap()

    cstf = nc.inline_tensor(_const_f32(), name="cstf").ap()

    MAGIC = 1.5 * 2 ** 23
    TWO_PI = 2.0 * PI

    with TileContext(nc) as tc:
        with (
            tc.tile_pool(name="wpool", bufs=1) as wpool,
            tc.tile_pool(name="sm", bufs=2) as sm,
            tc.tile_pool(name="db", bufs=2) as db,
            tc.tile_pool(name="xp", bufs=4) as xp,
            tc.tile_pool(name="cv", bufs=4) as cv,
            tc.tile_pool(name="psA", bufs=1, space="PSUM") as psA,
            tc.tile_pool(name="psB", bufs=1, space="PSUM") as psB,
            tc.tile_pool(name="psS", bufs=1, space="PSUM") as psS,
            tc.tile_pool(name="psO", bufs=1, space="PSUM") as psO,
        ):
            # ---- t=0: dep-free memsets, act-table steering, PE warm-up
            zsrc = sm.tile([1, 1], f32, tag="zsrc")
            nc.vector.memset(zsrc[:], 0.0)
            # GPSIMD's queue starts ~1.3us before Vector's: memset the
            # warm-chain slice there so the PE warm-up begins ~6.4us and
            # the queue is clear before the first real matmul's operands
            wdum = sm.tile([128, 512], bf16, tag="wdum")
            nc.gpsimd.memset(wdum[:, 0:64], 0.0)
            nc.vector.memset(wdum[:, 64:512], 0.0)
            sdum = sm.tile([1, 1], f32, tag="sdum")
            # steers the act-table pass: silu_and_others serves
            # Tanh+Sin+Copy+Square, so only one ACT_TABLE_LOAD is emitted
            nc.scalar.activation(sdum[:], zsrc[:], AF.Silu)

            # warm-up: WAW-chained dummy matmuls release the HAM clock gate
            # (~3.4us of sustained PE busy) before the real matmuls start;
            # later heartbeat() calls refill PE-idle gaps so the free-running
            # HAM windows never see enough idle to re-throttle
            wps = psA.tile([1, 512], f32, tag="y0")
            for _ in range(WARM_MM):
                nc.tensor.matmul(wps[:, 0:64], wdum[:, 0:1],
                                 wdum[:, 0:64],
                                 start=True, stop=True,
                                 skip_group_check=True)
            hps = psO.tile([1, 512], f32, tag="o1B")

            def heartbeat(n=1):
                for _ in range(n):
                    nc.tensor.matmul(hps[:], wdum[:, 0:1], wdum[:],
                                     start=True, stop=True,
                                     skip_group_check=True)

            # ---- DMAs: input + 2 const packs
            sx = sm.tile([64, 10], f32, tag="sx")
            nc.sync.dma_start(sx[:], xin, single_packet=True)
            ctf = wpool.tile([128, NF_F32], f32, tag="cstf")
            nc.sync.dma_start(ctf[:, 0:80], cstf[:, 0:80])
            nc.sync.dma_start(ctf[:, 80:208], cstf[:, 80:208])
            nc.sync.dma_start(ctf[:, 208:NF_F32], cstf[:, 208:NF_F32])
            wt = wpool.tile([128, NB_W], bf16, tag="wt")
            nc.scalar.dma_start(wt[:], wpack)

            cmat = ctf[:, 2:6]
            ident4 = ctf[0:4, 6:10]
            scol = ctf[0:64, 10:12]
            ci64 = ctf[0:64, 16:80]
            ctwb65 = ctf[0:65, 80:208]
            umask = ctf[0:64, 208:240]
            ca_t = ctf[0:32, 240:245]
            w1_t = ctf[:, 248:264].bitcast(bf16)

            def W(m):
                return wt[:, 128 * m:128 * m + 128]

            # ---- feature map: tanh, diagonal spread (+ ones row), one
            # matmul per chunk -> omega*t/2pi + bias-in-turns
            tx = sm.tile([64, 10], f32, tag="tx")
            nc.scalar.activation(tx[:], sx[:], AF.Tanh)
            spr = sm.tile([65, 640], f32, tag="spr")
            nc.vector.memset(spr[64:65, :], 1.0)
            nc.vector.tensor_tensor(
                spr[0:64].bitcast(f32r).rearrange("p (q b) -> p q b", b=64),
                tx[:].unsqueeze(2).broadcast_to((64, 10, 64)),
                ci64.unsqueeze(1).broadcast_to((64, 10, 64)),
                OP.mult)
            argp = [None, None]
            for i in range(2):
                sl = slice(320 * i, 320 * (i + 1))
                ap_ = psA.tile([128, 320], f32, tag=f"y{i}")
                nc.tensor.matmul(ap_[:], ctwb65.bitcast(f32r),
                                 spr[:, sl].bitcast(f32r),
                                 start=True, stop=True)
                argp[i] = ap_

            heartbeat(3)
            trig = sm.tile([128, 640], f32, tag="trig")
            for i in range(2):
                sl = slice(320 * i, 320 * (i + 1))
                kk = sm.tile([128, 320], f32, tag=f"kk{i}")
                if FUSED_ROUND:
                    nc.vector.tensor_scalar(kk[:], argp[i][:], MAGIC, MAGIC,
                                            OP.add, OP.subtract)
                else:
                    nc.vector.tensor_scalar(kk[:], argp[i][:], MAGIC, None,
                                            OP.add)
                    nc.vector.tensor_scalar(kk[:], kk[:], MAGIC, None,
                                            OP.subtract)
                ar = sm.tile([128, 320], f32, tag=f"ar{i}")
                nc.vector.scalar_tensor_tensor(ar[:], kk[:], -1.0,
                                               argp[i][:], OP.mult, OP.add)
                nc.scalar.activation(trig[:, sl].bitcast(f32r), ar[:],
                                     AF.Sin, scale=TWO_PI)

            # ---- v = C^T trig : 2 matmuls then 10 tiny PE transposes
            # (column order QORD so the kron tree reads grouped pairs)
            vp0 = psS.tile([4, 320], f32, tag="vp0")
            vp1 = psS.tile([4, 320], f32, tag="vp1")
            vps = [vp0, vp1]
            for i in range(2):
                nc.tensor.matmul(vps[i][:], cmat.bitcast(f32r),
                                 trig[:, 320 * i:320 * (i + 1)].bitcast(f32r),
                                 start=True, stop=True)
            heartbeat(2)
            vsb = sm.tile([4, 640], f32, tag="vsb")
            nc.scalar.activation(vsb[:, 0:320].bitcast(f32r), vp0[:],
                                 AF.Copy)
            nc.vector.tensor_copy(vsb[:, 320:640].bitcast(f32r), vp1[:])
            vT = psS.tile([64, 40], f32, tag="vp0")
            for k, q in enumerate(QORD):
                nc.tensor.transpose(
                    vT[:, 4 * k:4 * k + 4].bitcast(f32r),
                    vsb[0:4, 64 * q:64 * q + 64].bitcast(f32r),
                    ident4.bitcast(f32r))
            v_cur = sm.tile([64, 40], f32, tag="vcur")
            nc.vector.tensor_copy(v_cur[:], vT[:])

            # PE bridge across the DVE-only kron-tree stretch
            heartbeat(6)

            # ---- fused kron tree, planar re/im (DVE APs are max 3 free
            # dims, so grouped complex products use 4 mults + 2 adds)
            # P1 planes: (g, m, n); pairs g = (QORD[g], QORD[5+g])
            V = nc.vector
            vv = v_cur[:].rearrange("p (k a c) -> p k a c", a=2, c=2)
            Xre = vv[:, 0:5, :, 0]     # [64, 5, 2(m)]
            Xim = vv[:, 0:5, :, 1]
            Yre = vv[:, 5:10, :, 0]    # [64, 5, 2(n)]
            Yim = vv[:, 5:10, :, 1]
            trr = db.tile([64, 20], f32, tag="trr")
            tii = db.tile([64, 20], f32, tag="tii")
            tri = db.tile([64, 20], f32, tag="tri")
            tir = db.tile([64, 20], f32, tag="tir")
            P1 = db.tile([64, 40], f32, tag="P1")
            P1re, P1im = P1[:, 0:20], P1[:, 20:40]

            def _mul(out, a, b, g, m, n):
                V.tensor_tensor(
                    out[:].rearrange("p (g m n) -> p g m n", m=m, n=n),
                    a.unsqueeze(3).broadcast_to((64, g, m, n)),
                    b.unsqueeze(2).broadcast_to((64, g, m, n)),
                    OP.mult)

            _mul(trr, Xre, Yre, 5, 2, 2)
            _mul(tii, Xim, Yim, 5, 2, 2)
            _mul(tri, Xre, Yim, 5, 2, 2)
            _mul(tir, Xim, Yre, 5, 2, 2)
            V.tensor_tensor(P1re, trr[:], tii[:], OP.subtract)
            V.tensor_tensor(P1im, tri[:], tir[:], OP.add)

            # L2: k=0: p50 (x) p12 -> (q5,q0,q1,q2); k=1: p67 (x) p89 -> l4
            X2re = P1re[:, 0:16].rearrange("p (k r) -> p k r", r=8)[:, :, 0:4]
            X2im = P1im[:, 0:16].rearrange("p (k r) -> p k r", r=8)[:, :, 0:4]
            Y2re = P1re[:, 0:16].rearrange("p (k r) -> p k r", r=8)[:, :, 4:8]
            Y2im = P1im[:, 0:16].rearrange("p (k r) -> p k r", r=8)[:, :, 4:8]
            urr = db.tile([64, 32], f32, tag="urr")
            uii = db.tile([64, 32], f32, tag="uii")
            uri = db.tile([64, 32], f32, tag="uri")
            uir = db.tile([64, 32], f32, tag="uir")
            P2 = db.tile([64, 64], f32, tag="P2")
            P2re, P2im = P2[:, 0:32], P2[:, 32:64]
            _mul(urr, X2re, Y2re, 2, 4, 4)
            _mul(uii, X2im, Y2im, 2, 4, 4)
            _mul(uri, X2re, Y2im, 2, 4, 4)
            _mul(uir, X2im, Y2re, 2, 4, 4)
            V.tensor_tensor(P2re, urr[:], uii[:], OP.subtract)
            V.tensor_tensor(P2im, uri[:], uir[:], OP.add)

            # L3: h6 = p5012 (x) p34, planar again -> s1bf c-major means
            # re plane then im plane, so both adds write contiguously
            X3re, X3im = P2re[:, 0:16], P2im[:, 0:16]
            p34re, p34im = P1re[:, 16:20], P1im[:, 16:20]
            hrr = db.tile([64, 64], f32, tag="hrr")
            hii = db.tile([64, 64], f32, tag="hii")
            hri = db.tile([64, 64], f32, tag="hri")
            hir = db.tile([64, 64], f32, tag="hir")

            def _mul3(out, a, b):
                V.tensor_tensor(
                    out[:].rearrange("p (m n) -> p m n", n=4),
                    a.unsqueeze(2).broadcast_to((64, 16, 4)),
                    b.unsqueeze(1).broadcast_to((64, 16, 4)),
                    OP.mult)

            _mul3(hrr, X3re, p34re)
            _mul3(hii, X3im, p34im)
            _mul3(hri, X3re, p34im)
            _mul3(hir, X3im, p34re)
            s1bf = sm.tile([64, 128], bf16, tag="s1bf")
            V.tensor_tensor(s1bf[:, 0:64], hrr[:], hii[:], OP.subtract)
            V.tensor_tensor(s1bf[:, 64:128], hri[:], hir[:], OP.add)

            # ---- S2 from S1 (c-major): S2[c] = sign(c) * S1[1-c]
            s1cm = s1bf[:].rearrange("p (c j) -> p c j", c=2)
            s2bf = sm.tile([64, 128], bf16, tag="s2bf")
            nc.vector.tensor_tensor(
                s2bf[:].rearrange("p (c j) -> p c j", c=2),
                s1cm[:, ::-1, :],
                scol.unsqueeze(2).broadcast_to((64, 2, 64)),
                OP.mult)

            # m_ts free order (u4, j4, b0): b0 innermost rides the int32
            # pair-transposes of the layer loop
            m_ts = []
            for ci in range(2):
                l4ci = P2[:, 32 * ci + 16:32 * ci + 32]  # plane ci, k=1
                mbf = sm.tile([64, 512], bf16, tag=f"m{ci}bf")
                mv = mbf[:].rearrange("p (u j b) -> p u j b", u=16, b=2)
                for b0 in range(2):
                    nc.vector.tensor_tensor(
                        mv[:, :, :, b0],
                        l4ci.unsqueeze(1).broadcast_to((64, 16, 16)),
                        umask[:, 16 * b0:16 * b0 + 16].unsqueeze(2)
                            .broadcast_to((64, 16, 16)),
                        OP.mult)
                m_ts.append(mbf)

            # ---- X build: per half, 2 accumulated matmuls
            xa = []
            for h in range(2):
                psX = psA.tile([128, 512], f32, tag=f"y{h}")
                nc.tensor.matmul(psX[:],
                                 s1bf[32 * h:32 * h + 32],
                                 m_ts[0][32 * h:32 * h + 32, :],
                                 start=True, stop=False)
                nc.tensor.matmul(psX[:],
                                 s2bf[32 * h:32 * h + 32, :],
                                 m_ts[1][32 * h:32 * h + 32, :],
                                 start=False, stop=True)
                xt = xp.tile([128, 512], bf16, tag=f"x{h}")
                if h == 0:
                    nc.scalar.activation(xt[:], psX[:], AF.Copy)
                else:
                    nc.vector.tensor_copy(xt[:], psX[:])
                xa.append(xt)

            # ---- layers: matmul -> stream-transpose straight from PSUM
            # measurement-front tiles declared up front so the squares +
            # W1 matmul for each half can be emitted inside the loop at
            # l == 3 (the scheduler then starts them as soon as that
            # half's final state lands instead of after the whole loop)
            sq = sm.tile([128, 1024], bf16, tag="sq")
            sqB = sm.tile([128, 1024], bf16, tag="sqB")
            sqB_j = sqB[:].rearrange("p (j b g) -> p j b g",
                                     j=32, b=2, g=16)
            o1A = [None, None]

            def meas_front(h, zbh):
                zv = zbh[:].rearrange("p (u j b) -> p u b j", u=8, b=2)
                nc.scalar.square(
                    sq[:, 512 * h:512 * h + 512]
                        .rearrange("p (u b j) -> p u b j", u=8, b=2), zv)
                nc.scalar.square(
                    sqB_j[:, :, :, 8 * h:8 * h + 8],
                    zbh[:].rearrange("p (u j b) -> p j b u", u=8, b=2))
                oA = psA.tile([32, 512], f32, tag=f"y{h}")
                nc.tensor.matmul(oA[:], w1_t,
                                 sq[:, 512 * h:512 * h + 512],
                                 start=True, stop=True)
                o1A[h] = oA

            zb = [None, None]
            gdum = sm.tile([1, 1], f32, tag="gdum")
            # wake GPSIMD mid-kernel (reads a never-rewritten tile so the
            # slow wake-up cannot stall the layer pipeline via WAR deps)
            nc.gpsimd.tensor_copy(gdum[:], s1bf[0:1, 0:1])
            xa_mv = [xa[0][:], xa[1][:]]
            for l in range(N_LAYERS):
                for h in range(2):
                    yA = psA.tile([128, 512], f32, tag=f"y{h}")
                    nc.tensor.matmul(yA[:], W(2 * l), xa_mv[h],
                                     start=True, stop=True)
                    if TR_FROM_PSUM:
                        # StreamTranspose needs same src/dst dtype: f32r in
                        # (bitcast of the f32 PSUM), f32r-rounded out
                        xB = cv.tile([128, 512], f32, tag=f"xb{h}")
                        nc.vector.transpose(xB[:], yA[:])
                        xB_mv = xB[:]
                    else:
                        yc = cv.tile([128, 512], bf16, tag=f"yc{h}")
                        nc.scalar.activation(yc[:], yA[:], AF.Copy)
                        xBb = cv.tile([128, 512], bf16, tag=f"xb{h}")
                        nc.vector.transpose(xBb[:].bitcast(u32),
                                            yc[:].bitcast(u32))
                        xB_mv = xBb[:]
                    zB = psB.tile([128, 512], f32, tag=f"z{h}")
                    nc.tensor.matmul(zB[:], W(2 * l + 1), xB_mv,
                                     start=True, stop=True)
                    if l < N_LAYERS - 1:
                        if TR_FROM_PSUM:
                            xt = xp.tile([128, 512], f32, tag=f"x{h}")
                            nc.vector.transpose(xt[:], zB[:])
                            xa_mv[h] = xt[:]
                        else:
                            zc = cv.tile([128, 512], bf16, tag=f"zc{h}")
                            nc.scalar.activation(zc[:], zB[:], AF.Copy)
                            xt = xp.tile([128, 512], bf16, tag=f"x{h}")
                            nc.vector.transpose(xt[:].bitcast(u32),
                                                zc[:].bitcast(u32))
                            xa_mv[h] = xt[:]
                    else:
                        zb[h] = zB
                        meas_front(h, zB)

            # ---- measurement (fronts already emitted at l == 3)
            # sq stored as (h, u3, b0, j5); sqB as (j5, b0, h, u3)
            o1B = psO.tile([32, 1024], f32, tag="o1B")
            for g in range(2):
                nc.tensor.matmul(o1B[:, 512 * g:512 * g + 512], w1_t,
                                 sqB[:, 512 * g:512 * g + 512],
                                 start=True, stop=True)

            # o1t[j5, (h, u, (b0,o))] and o1u[(h,u), (j5, (b0,o))]
            o1t = sm.tile([32, 1024], f32, tag="o1t")
            for h in range(2):
                nc.vector.transpose(o1t[:, 512 * h:512 * h + 512],
                                    o1A[h][:])
            # TR2 split at the j5 midpoint: each half transposes as soon
            # as its o1B matmul lands instead of waiting for both
            o1u = sm.tile([32, 1024], f32, tag="o1u")
            for g in range(2):
                nc.vector.transpose(o1u[:, 512 * g:512 * g + 512],
                                    o1B[:, 512 * g:512 * g + 512])

            # q0..4: stationary-data matmul -> [64, 5] PSUM. Stationary
            # free emitted in natural (g, b0, b1) order (collapses to one
            # AP dim); rows land as r = (g, b0, b1) -- the host gather
            # un-swaps b0/b1 (ROWPERM)
            sa = o1t[:].rearrange("p (gb o) -> p gb o", o=16)[:, :, 0]
            outa = psS.tile([64, 5], f32, tag="vp1")
            nc.tensor.matmul(outa[:], sa, ca_t, start=True, stop=True)
            outa_sb = sm.tile([64, 5], f32, tag="outa")
            nc.vector.tensor_copy(outa_sb[:], outa[:])

            # q5..9: two partial strided reduces (one per o1u half, each
            # ready right after its transpose) + a tiny add
            out59p = sm.tile([32, 20], f32, tag="out59p")
            for g in range(2):
                nc.vector.tensor_reduce(
                    out59p[:, 10 * g:10 * g + 10]
                        .rearrange("p (b o) -> p b o", b=2),
                    o1u[:, 512 * g:512 * g + 512]
                        .rearrange("p (j b o) -> p b o j",
                                   j=16, b=2, o=16)[:, :, 1:6, :],
                    mybir.AxisListType.X, OP.add)
            out59 = sm.tile([32, 10], f32, tag="out59")
            nc.vector.tensor_tensor(out59[:], out59p[:, 0:10],
                                    out59p[:, 10:20], OP.add)

            # second GPSIMD touch near the tail so the postamble's
            # semaphore range-clear does not pay a wake-up
            nc.gpsimd.tensor_copy(gdum[:], sq[0:1, 0:1])
            # q0..4: psum partitions already in sample-row order
            nc.sync.dma_start(out_d[:, 0:5], outa_sb[:])
            # q5..9: partition p = (b0, (h,u3)); write rows in the same
            # swapped convention r = 4k + 2*b0 + b1 the host un-permutes
            o59v = out59[:].rearrange("p (b1 o) -> p b1 o", b1=2)
            outv = out_d.rearrange("(k b0 b1) q -> k b0 b1 q", b0=2, b1=2)
            for b0 in range(2):
                eng = nc.sync if b0 == 0 else nc.scalar
                eng.dma_start(outv[:, b0, :, 5:10],
                              o59v[16 * b0:16 * b0 + 16])

    nc.finalize()
    return nc


def _get_module():
    if "nc" not in _BUILD_CACHE:
        _BUILD_CACHE["nc"] = _build_module()
    return _BUILD_CACHE["nc"]


# ---------------------------------------------------------------- entrypoint
def kernel(inputs, theta):
    inputs = np.asarray(inputs, dtype=np.float32)
    theta = np.asarray(theta, dtype=np.float32)
    assert inputs.shape == (B_TOTAL, N_QUBITS)

    from concourse.bass_utils import run_bass_kernel_spmd

    nc = _get_module()
    wpack = _pack_w(theta)
    in_maps = []
    for c in range(N_CORES):
        shard = np.ascontiguousarray(inputs[B_CORE * c:B_CORE * (c + 1)])
        in_maps.append({"xin": shard, "wpack": wpack})
    res = run_bass_kernel_spmd(nc, in_maps, core_ids=list(range(N_CORES)))
    out = np.empty((B_TOTAL, N_QUBITS), np.float32)
    for c in range(N_CORES):
        out[B_CORE * c + ROWPERM] = res.results[c]["out"]
    return out


# revision 72
# speedup vs baseline: 1.1734x; 1.1734x over previous
"""Trainium2 Bass kernel v9 for nn_EnhancedQuantumLayer (10-qubit, 4-layer
variational circuit, batch 512, Z-expectations output).

Data parallel over 8 cores, 64 samples/core. Changes vs the v2 baseline
(58.3us -> ~47us):
  - PE warm-up chain (~4.3us of dep-free dummy matmuls) releases the HAM
    clock gate before the real matmuls; data-pinned heartbeat matmuls
    refill later PE-idle stretches (HAM re-throttles on idle windows).
  - Act-table steering Silu reads a memset tile (no DMA dependency) so
    it schedules first on ACT and one table set serves tanh/sin/sq/copy.
  - Consts consolidated: one f32 pack (3 pipelined DMAs) + the bf16
    weight stack; the 160KB dmask is gone - the cos bias rides a
    ones-row in the spread so one matmul per chunk gives
    omega*t/2pi + bias-in-turns.
  - Range reduction: fused magic-round tensor_scalar (+M, -M) + one STT;
    sin(2pi*diff) via the activation scale field.
  - Doubling chain fused into a 3-level planar (re/im) kron tree,
    ~19 DVE ops instead of ~35, via the QORD column layout.
  - Batch bit b0 kept innermost in both layouts so the layer-loop
    32x32 block transposes run on uint32 pairs (444ns vs 712ns); the
    B-space op embeds identity on b1 instead of b0 (same matrix).
  - Measurement: dual-permutation squares feed plain-moving W1 matmuls;
    one stream transpose + stationary-data matmul gives q0..4 in
    sample partitions, another + strided reduce gives q5..9; output is
    3 clean DMAs (rows come back (b>>2, b0, b1)-ordered; host applies
    ROWPERM when gathering).
  - GPSIMD touched mid-kernel and near the tail so the postamble
    semaphore range-clear pays no wake-up.

Host precompute is theta-only: 8 realified 128x128 stationaries (bf16).
"""

import math

import ml_dtypes
import numpy as np

N_QUBITS = 10
N_LAYERS = 4
FREQS = (1.0, 2.0, 4.0, 8.0, 16.0)
PI = float(np.pi)
B_TOTAL = 512
B_CORE = 64
N_CORES = 8
WARM_MM = 26
FUSED_ROUND = True
TR_FROM_PSUM = False

CZCNOT = np.array([[1, 0, 0, 0],
                   [0, 1, 0, 0],
                   [0, 0, 0, -1],
                   [0, 0, 1, 0]], dtype=np.complex128)

# vT column order: L1 kron pairs are g = (QORD[g], QORD[5+g]), laid out
# as (5,0) (1,2) (6,7) (8,9) (3,4) so the L2 operands p50/p67 (X) and
# p12/p89 (Y) sit at regular 16-col strides and p34 is contiguous.
QORD = (5, 1, 6, 8, 3, 0, 2, 7, 9, 4)

# device output rows come back as r = (b>>2, b0, b1); ROWPERM[r] = b
ROWPERM = np.array([(r & ~3) + 2 * (r & 1) + ((r >> 1) & 1)
                    for r in range(B_CORE)])


# ---------------------------------------------------------------- host math
def _rz(phi):
    return np.array([[np.exp(-0.5j * phi), 0], [0, np.exp(0.5j * phi)]])


def _rx(th):
    c, s = np.cos(th / 2), np.sin(th / 2)
    return np.array([[c, -1j * s], [-1j * s, c]])


def _ry(th):
    c, s = np.cos(th / 2), np.sin(th / 2)
    return np.array([[c, -s], [s, c]])


def _kron_list(ms):
    out = ms[0]
    for m in ms[1:]:
        out = np.kron(out, m)
    return out


def _embed_2q(space_qubits, qa, qb, M4):
    n = len(space_qubits)
    dim = 2 ** n
    pa, pb = space_qubits.index(qa), space_qubits.index(qb)
    out = np.zeros((dim, dim), dtype=np.complex128)
    for idx in range(dim):
        bits = [(idx >> (n - 1 - i)) & 1 for i in range(n)]
        col4 = 2 * bits[pa] + bits[pb]
        for row4 in range(4):
            val = M4[row4, col4]
            if val != 0:
                nb = bits.copy()
                nb[pa], nb[pb] = row4 >> 1, row4 & 1
                ridx = sum(bit << (n - 1 - i) for i, bit in enumerate(nb))
                out[ridx, idx] += val
    return out


def _realify(M):
    return np.block([[M.real, -M.imag], [M.imag, M.real]])


def _embed_OB(M_L):
    """layout-B partition op on (q5, b0, q6..q9): identity on b0."""
    M = M_L.reshape(2, 16, 2, 16)
    O = np.zeros((2, 2, 16, 2, 2, 16), np.complex128)
    for b0 in range(2):
        O[:, b0, :, :, b0, :] = M
    return O.reshape(64, 64)


def _host_weights(theta):
    """wstack [128, 8, 128] bf16: per layer [lhsT_A, lhsT_L] with
    lhsT = realify(op).T, partition-major for one contiguous DMA."""
    ang = np.tanh(theta.astype(np.float64)) * PI
    A_space = [5, 0, 1, 2, 3, 4]
    L_space = [5, 6, 7, 8, 9]
    mats = []
    for l in range(N_LAYERS):
        U = []
        for q in range(10):
            a0, a1, a2 = ang[l, q]
            U.append(_rx(a0 * 0.5) @ _rz(a2) @ _ry(a1) @ _rz(a0))
        UA = _kron_list([U[q] for q in A_space])
        E_even_A = (_embed_2q(A_space, 0, 1, CZCNOT)
                    @ _embed_2q(A_space, 2, 3, CZCNOT)
                    @ _embed_2q(A_space, 4, 5, CZCNOT))
        E_odd_A = (_embed_2q(A_space, 1, 2, CZCNOT)
                   @ _embed_2q(A_space, 3, 4, CZCNOT))
        M_A = E_odd_A @ E_even_A @ UA
        UL = _kron_list([np.eye(2)] + [U[q] for q in [6, 7, 8, 9]])
        E_even_L = (_embed_2q(L_space, 6, 7, CZCNOT)
                    @ _embed_2q(L_space, 8, 9, CZCNOT))
        E_odd_L = (_embed_2q(L_space, 5, 6, CZCNOT)
                   @ _embed_2q(L_space, 7, 8, CZCNOT))
        M_L = E_odd_L @ E_even_L @ UL
        mats.append(_realify(M_A).T)
        mats.append(_realify(_embed_OB(M_L)).T)
    stk = np.stack(mats)  # [8, 128, 128]
    return np.ascontiguousarray(stk.transpose(1, 0, 2))  # [128, 8, 128] f64


# ------------------------------------------------------- fourier basis (v)
def _v_of_t(t):
    t = np.atleast_1d(np.asarray(t, np.float64))
    v = np.zeros((t.size, 2), np.complex128)
    v[:, 0] = 1.0
    for f in FREQS:
        phi = f * t
        v = v * np.stack([np.exp(-0.5j * phi), np.exp(0.5j * phi)], -1)
        th = 0.25 * f * t
        c, s = np.cos(th), np.sin(th)
        v = np.stack([c * v[:, 0] - 1j * s * v[:, 1],
                      -1j * s * v[:, 0] + c * v[:, 1]], -1)
    return v


def _fourier_C():
    """C [94, 4]: rows 0-46 sin(0.25 m t), rows 47-93 cos, m = 1,3..93;
    comps (ar, ai, br, bi)."""
    N = 1024
    ts = np.arange(N) * (8 * np.pi / N)
    vv = _v_of_t(ts)
    comps = np.stack([vv[:, 0].real, vv[:, 0].imag,
                      vv[:, 1].real, vv[:, 1].imag], -1)
    F = np.fft.rfft(comps, axis=0)
    msk = np.arange(1, 94, 2)
    a_cos = 2.0 * F[msk].real / N
    b_sin = -2.0 * F[msk].imag / N
    return msk, np.concatenate([b_sin, a_cos], 0).astype(np.float64)


# ------------------------------------------------------------- const packs
def _w1_matrix():
    # W1 [128, 32]: col = b0*16 + o; o: 0 = plain sum, 1..5 = s(q5, q6..9)
    W1 = np.zeros((128, 32), np.float32)
    for p in range(128):
        q5 = (p >> 5) & 1
        b0 = (p >> 4) & 1
        j4 = p & 15
        s = [1 - 2 * q5] + [1 - 2 * ((j4 >> (3 - k)) & 1) for k in range(4)]
        W1[p, b0 * 16 + 0] = 1.0
        for k in range(5):
            W1[p, b0 * 16 + 1 + k] = s[k]
    return W1


def _ca_matrix():
    # Ca [32, 5]: sign of bit q (q0 = MSB of j5) for q = 0..4
    Ca = np.zeros((32, 5), np.float32)
    for j in range(32):
        for q in range(5):
            Ca[j, q] = 1 - 2 * ((j >> (4 - q)) & 1)
    return Ca


NF_F32 = 280


def _const_f32():
    msk, C94 = _fourier_C()
    cf = np.zeros((128, NF_F32), np.float32)
    cf[0:94, 2:6] = C94
    cf[0:4, 6:10] = np.eye(4)
    cf[0:64, 10] = -1.0
    cf[0:64, 11] = 1.0
    cf[0:64, 16:80] = np.eye(64)
    # omega/bias stationary [65, 128]: rows 0..63 = om/2pi, row 64 = bias/2pi
    om = np.zeros(128, np.float64)
    om[0:47] = 0.25 * msk
    om[47:94] = 0.25 * msk
    cf[0:64, 80:208] = (om / (2 * np.pi)).astype(np.float32)
    bias = np.zeros(128, np.float32)
    bias[47:94] = 0.25  # pi/2 in turns
    cf[64, 80:208] = bias
    # umask2 [64, 32]: col = b0*16 + u4; delta(u4 = (b%32)>>1, b0 = b&1)
    for b in range(64):
        cf[b, 208 + (b & 1) * 16 + ((b % 32) >> 1)] = 1.0
    cf[0:32, 240:245] = _ca_matrix()
    # W1 [128, 32] bf16 bit-packed into f32 cols 248:264
    w1b = _w1_matrix().astype(ml_dtypes.bfloat16).view(np.uint16)
    cfu = cf.view(np.uint32)
    cfu[:, 248:264] = w1b[:, 0::2].astype(np.uint32) | (
        w1b[:, 1::2].astype(np.uint32) << 16)
    return cf


NB_W = 1024


def _pack_w(theta):
    wstack = _host_weights(theta)  # [128, 8, 128] f64
    return np.ascontiguousarray(
        wstack.reshape(128, NB_W).astype(ml_dtypes.bfloat16))


# ------------------------------------------------------------- bass builder
_BUILD_CACHE = {}


def _build_module():
    import concourse.bass as bass
    import concourse.mybir as mybir
    from concourse import bacc
    from concourse.tile import TileContext

    f32 = mybir.dt.float32
    f32r = mybir.dt.float32r
    bf16 = mybir.dt.bfloat16
    u32 = mybir.dt.uint32
    AF = mybir.ActivationFunctionType
    OP = mybir.AluOpType

    nc = bacc.Bacc("TRN2", target_bir_lowering=False, debug=False)

    xin = nc.dram_tensor("xin", [B_CORE, 10], f32, kind="ExternalInput").ap()
    wpack = nc.dram_tensor("wpack", [128, NB_W], bf16,
                           kind="ExternalInput").ap()
    out_d = nc.dram_tensor("out", [B_CORE, 10], f32, kind="ExternalOutput").ap()

    cstf = nc.inline_tensor(_const_f32(), name="cstf").ap()

    MAGIC = 1.5 * 2 ** 23
    TWO_PI = 2.0 * PI

    with TileContext(nc) as tc:
        with (
            tc.tile_pool(name="wpool", bufs=1) as wpool,
            tc.tile_pool(name="sm", bufs=2) as sm,
            tc.tile_pool(name="db", bufs=2) as db,
            tc.tile_pool(name="xp", bufs=4) as xp,
            tc.tile_pool(name="cv", bufs=4) as cv,
            tc.tile_pool(name="psA", bufs=1, space="PSUM") as psA,
            tc.tile_pool(name="psB", bufs=1, space="PSUM") as psB,
            tc.tile_pool(name="psS", bufs=1, space="PSUM") as psS,
            tc.tile_pool(name="psO", bufs=1, space="PSUM") as psO,
        ):
            # ---- t=0: dep-free memsets, act-table steering, PE warm-up
            zsrc = sm.tile([1, 1], f32, tag="zsrc")
            nc.vector.memset(zsrc[:], 0.0)
            # GPSIMD's queue starts ~1.3us before Vector's: memset the
            # warm-chain slice there so the PE warm-up begins ~6.4us and
            # the queue is clear before the first real matmul's operands
            wdum = sm.tile([128, 512], bf16, tag="wdum")
            nc.gpsimd.memset(wdum[:, 0:64], 0.0)
            nc.vector.memset(wdum[:, 64:512], 0.0)
            sdum = sm.tile([1, 1], f32, tag="sdum")
            # steers the act-table pass: silu_and_others serves
            # Tanh+Sin+Copy+Square, so only one ACT_TABLE_LOAD is emitted
            nc.scalar.activation(sdum[:], zsrc[:], AF.Silu)

            # warm-up: WAW-chained dummy matmuls release the HAM clock gate
            # (~3.4us of sustained PE busy) before the real matmuls start;
            # later heartbeat() calls refill PE-idle gaps so the free-running
            # HAM windows never see enough idle to re-throttle
            wps = psA.tile([1, 512], f32, tag="y0")
            for _ in range(WARM_MM):
                nc.tensor.matmul(wps[:, 0:64], wdum[:, 0:1],
                                 wdum[:, 0:64],
                                 start=True, stop=True,
                                 skip_group_check=True)
            hps = psO.tile([1, 512], f32, tag="o1B")

            def heartbeat(n=1):
                for _ in range(n):
                    nc.tensor.matmul(hps[:], wdum[:, 0:1], wdum[:],
                                     start=True, stop=True,
                                     skip_group_check=True)

            # ---- DMAs: input + 2 const packs
            sx = sm.tile([64, 10], f32, tag="sx")
            nc.sync.dma_start(sx[:], xin, single_packet=True)
            ctf = wpool.tile([128, NF_F32], f32, tag="cstf")
            nc.sync.dma_start(ctf[:, 0:80], cstf[:, 0:80])
            nc.sync.dma_start(ctf[:, 80:208], cstf[:, 80:208])
            nc.sync.dma_start(ctf[:, 208:NF_F32], cstf[:, 208:NF_F32])
            wt = wpool.tile([128, NB_W], bf16, tag="wt")
            nc.scalar.dma_start(wt[:], wpack)

            cmat = ctf[:, 2:6]
            ident4 = ctf[0:4, 6:10]
            scol = ctf[0:64, 10:12]
            ci64 = ctf[0:64, 16:80]
            ctwb65 = ctf[0:65, 80:208]
            umask = ctf[0:64, 208:240]
            ca_t = ctf[0:32, 240:245]
            w1_t = ctf[:, 248:264].bitcast(bf16)

            def W(m):
                return wt[:, 128 * m:128 * m + 128]

            # ---- feature map: tanh, diagonal spread (+ ones row), one
            # matmul per chunk -> omega*t/2pi + bias-in-turns
            tx = sm.tile([64, 10], f32, tag="tx")
            nc.scalar.activation(tx[:], sx[:], AF.Tanh)
            spr = sm.tile([65, 640], f32, tag="spr")
            nc.vector.memset(spr[64:65, :], 1.0)
            nc.vector.tensor_tensor(
                spr[0:64].bitcast(f32r).rearrange("p (q b) -> p q b", b=64),
                tx[:].unsqueeze(2).broadcast_to((64, 10, 64)),
                ci64.unsqueeze(1).broadcast_to((64, 10, 64)),
                OP.mult)
            argp = [None, None]
            for i in range(2):
                sl = slice(320 * i, 320 * (i + 1))
                ap_ = psA.tile([128, 320], f32, tag=f"y{i}")
                nc.tensor.matmul(ap_[:], ctwb65.bitcast(f32r),
                                 spr[:, sl].bitcast(f32r),
                                 start=True, stop=True)
                argp[i] = ap_

            heartbeat(3)
            trig = sm.tile([128, 640], f32, tag="trig")
            for i in range(2):
                sl = slice(320 * i, 320 * (i + 1))
                kk = sm.tile([128, 320], f32, tag=f"kk{i}")
                if FUSED_ROUND:
                    nc.vector.tensor_scalar(kk[:], argp[i][:], MAGIC, MAGIC,
                                            OP.add, OP.subtract)
                else:
                    nc.vector.tensor_scalar(kk[:], argp[i][:], MAGIC, None,
                                            OP.add)
                    nc.vector.tensor_scalar(kk[:], kk[:], MAGIC, None,
                                            OP.subtract)
                ar = sm.tile([128, 320], f32, tag=f"ar{i}")
                nc.vector.scalar_tensor_tensor(ar[:], kk[:], -1.0,
                                               argp[i][:], OP.mult, OP.add)
                nc.scalar.activation(trig[:, sl].bitcast(f32r), ar[:],
                                     AF.Sin, scale=TWO_PI)

            # ---- v = C^T trig : 2 matmuls then 10 tiny PE transposes
            # (column order QORD so the kron tree reads grouped pairs)
            vp0 = psS.tile([4, 320], f32, tag="vp0")
            vp1 = psS.tile([4, 320], f32, tag="vp1")
            vps = [vp0, vp1]
            for i in range(2):
                nc.tensor.matmul(vps[i][:], cmat.bitcast(f32r),
                                 trig[:, 320 * i:320 * (i + 1)].bitcast(f32r),
                                 start=True, stop=True)
            heartbeat(2)
            vsb = sm.tile([4, 640], f32, tag="vsb")
            nc.scalar.activation(vsb[:, 0:320].bitcast(f32r), vp0[:],
                                 AF.Copy)
            nc.vector.tensor_copy(vsb[:, 320:640].bitcast(f32r), vp1[:])
            vT = psS.tile([64, 40], f32, tag="vp0")
            for k, q in enumerate(QORD):
                nc.tensor.transpose(
                    vT[:, 4 * k:4 * k + 4].bitcast(f32r),
                    vsb[0:4, 64 * q:64 * q + 64].bitcast(f32r),
                    ident4.bitcast(f32r))
            v_cur = sm.tile([64, 40], f32, tag="vcur")
            nc.vector.tensor_copy(v_cur[:], vT[:])

            # PE bridge across the DVE-only kron-tree stretch
            heartbeat(6)

            # ---- fused kron tree, planar re/im (DVE APs are max 3 free
            # dims, so grouped complex products use 4 mults + 2 adds)
            # P1 planes: (g, m, n); pairs g = (QORD[g], QORD[5+g])
            V = nc.vector
            vv = v_cur[:].rearrange("p (k a c) -> p k a c", a=2, c=2)
            Xre = vv[:, 0:5, :, 0]     # [64, 5, 2(m)]
            Xim = vv[:, 0:5, :, 1]
            Yre = vv[:, 5:10, :, 0]    # [64, 5, 2(n)]
            Yim = vv[:, 5:10, :, 1]
            trr = db.tile([64, 20], f32, tag="trr")
            tii = db.tile([64, 20], f32, tag="tii")
            tri = db.tile([64, 20], f32, tag="tri")
            tir = db.tile([64, 20], f32, tag="tir")
            P1 = db.tile([64, 40], f32, tag="P1")
            P1re, P1im = P1[:, 0:20], P1[:, 20:40]

            def _mul(out, a, b, g, m, n):
                V.tensor_tensor(
                    out[:].rearrange("p (g m n) -> p g m n", m=m, n=n),
                    a.unsqueeze(3).broadcast_to((64, g, m, n)),
                    b.unsqueeze(2).broadcast_to((64, g, m, n)),
                    OP.mult)

            _mul(trr, Xre, Yre, 5, 2, 2)
            _mul(tii, Xim, Yim, 5, 2, 2)
            _mul(tri, Xre, Yim, 5, 2, 2)
            _mul(tir, Xim, Yre, 5, 2, 2)
            V.tensor_tensor(P1re, trr[:], tii[:], OP.subtract)
            V.tensor_tensor(P1im, tri[:], tir[:], OP.add)

            # L2: k=0: p50 (x) p12 -> (q5,q0,q1,q2); k=1: p67 (x) p89 -> l4
            X2re = P1re[:, 0:16].rearrange("p (k r) -> p k r", r=8)[:, :, 0:4]
            X2im = P1im[:, 0:16].rearrange("p (k r) -> p k r", r=8)[:, :, 0:4]
            Y2re = P1re[:, 0:16].rearrange("p (k r) -> p k r", r=8)[:, :, 4:8]
            Y2im = P1im[:, 0:16].rearrange("p (k r) -> p k r", r=8)[:, :, 4:8]
            urr = db.tile([64, 32], f32, tag="urr")
            uii = db.tile([64, 32], f32, tag="uii")
            uri = db.tile([64, 32], f32, tag="uri")
            uir = db.tile([64, 32], f32, tag="uir")
            P2 = db.tile([64, 64], f32, tag="P2")
            P2re, P2im = P2[:, 0:32], P2[:, 32:64]
            _mul(urr, X2re, Y2re, 2, 4, 4)
            _mul(uii, X2im, Y2im, 2, 4, 4)
            _mul(uri, X2re, Y2im, 2, 4, 4)
            _mul(uir, X2im, Y2re, 2, 4, 4)
            V.tensor_tensor(P2re, urr[:], uii[:], OP.subtract)
            V.tensor_tensor(P2im, uri[:], uir[:], OP.add)

            # L3: h6 = p5012 (x) p34, planar again -> s1bf c-major means
            # re plane then im plane, so both adds write contiguously
            X3re, X3im = P2re[:, 0:16], P2im[:, 0:16]
            p34re, p34im = P1re[:, 16:20], P1im[:, 16:20]
            hrr = db.tile([64, 64], f32, tag="hrr")
            hii = db.tile([64, 64], f32, tag="hii")
            hri = db.tile([64, 64], f32, tag="hri")
            hir = db.tile([64, 64], f32, tag="hir")

            def _mul3(out, a, b):
                V.tensor_tensor(
                    out[:].rearrange("p (m n) -> p m n", n=4),
                    a.unsqueeze(2).broadcast_to((64, 16, 4)),
                    b.unsqueeze(1).broadcast_to((64, 16, 4)),
                    OP.mult)

            _mul3(hrr, X3re, p34re)
            _mul3(hii, X3im, p34im)
            _mul3(hri, X3re, p34im)
            _mul3(hir, X3im, p34re)
            s1bf = sm.tile([64, 128], bf16, tag="s1bf")
            V.tensor_tensor(s1bf[:, 0:64], hrr[:], hii[:], OP.subtract)
            V.tensor_tensor(s1bf[:, 64:128], hri[:], hir[:], OP.add)

            # ---- S2 from S1 (c-major): S2[c] = sign(c) * S1[1-c]
            s1cm = s1bf[:].rearrange("p (c j) -> p c j", c=2)
            s2bf = sm.tile([64, 128], bf16, tag="s2bf")
            nc.vector.tensor_tensor(
                s2bf[:].rearrange("p (c j) -> p c j", c=2),
                s1cm[:, ::-1, :],
                scol.unsqueeze(2).broadcast_to((64, 2, 64)),
                OP.mult)

            # m_ts free order (u4, j4, b0): b0 innermost rides the int32
            # pair-transposes of the layer loop
            m_ts = []
            for ci in range(2):
                l4ci = P2[:, 32 * ci + 16:32 * ci + 32]  # plane ci, k=1
                mbf = sm.tile([64, 512], bf16, tag=f"m{ci}bf")
                mv = mbf[:].rearrange("p (u j b) -> p u j b", u=16, b=2)
                for b0 in range(2):
                    nc.vector.tensor_tensor(
                        mv[:, :, :, b0],
                        l4ci.unsqueeze(1).broadcast_to((64, 16, 16)),
                        umask[:, 16 * b0:16 * b0 + 16].unsqueeze(2)
                            .broadcast_to((64, 16, 16)),
                        OP.mult)
                m_ts.append(mbf)

            # ---- X build: per half, 2 accumulated matmuls
            xa = []
            for h in range(2):
                psX = psA.tile([128, 512], f32, tag=f"y{h}")
                nc.tensor.matmul(psX[:],
                                 s1bf[32 * h:32 * h + 32],
                                 m_ts[0][32 * h:32 * h + 32, :],
                                 start=True, stop=False)
                nc.tensor.matmul(psX[:],
                                 s2bf[32 * h:32 * h + 32, :],
                                 m_ts[1][32 * h:32 * h + 32, :],
                                 start=False, stop=True)
                xt = xp.tile([128, 512], bf16, tag=f"x{h}")
                if h == 0:
                    nc.scalar.activation(xt[:], psX[:], AF.Copy)
                else:
                    nc.vector.tensor_copy(xt[:], psX[:])
                xa.append(xt)

            # ---- layers: matmul -> stream-transpose straight from PSUM
            # measurement-front tiles declared up front so the squares +
            # W1 matmul for each half can be emitted inside the loop at
            # l == 3 (the scheduler then starts them as soon as that
            # half's final state lands instead of after the whole loop)
            sq = sm.tile([128, 1024], bf16, tag="sq")
            sqB = sm.tile([128, 1024], bf16, tag="sqB")
            sqB_j = sqB[:].rearrange("p (j b g) -> p j b g",
                                     j=32, b=2, g=16)
            o1A = [None, None]

            def meas_front(h, zbh):
                zv = zbh[:].rearrange("p (u j b) -> p u b j", u=8, b=2)
                nc.scalar.square(
                    sq[:, 512 * h:512 * h + 512]
                        .rearrange("p (u b j) -> p u b j", u=8, b=2), zv)
                nc.scalar.square(
                    sqB_j[:, :, :, 8 * h:8 * h + 8],
                    zbh[:].rearrange("p (u j b) -> p j b u", u=8, b=2))
                oA = psA.tile([32, 512], f32, tag=f"y{h}")
                nc.tensor.matmul(oA[:], w1_t,
                                 sq[:, 512 * h:512 * h + 512],
                                 start=True, stop=True)
                o1A[h] = oA

            zb = [None, None]
            gdum = sm.tile([1, 1], f32, tag="gdum")
            # wake GPSIMD mid-kernel (reads a never-rewritten tile so the
            # slow wake-up cannot stall the layer pipeline via WAR deps)
            nc.gpsimd.tensor_copy(gdum[:], s1bf[0:1, 0:1])
            xa_mv = [xa[0][:], xa[1][:]]
            for l in range(N_LAYERS):
                for h in range(2):
                    yA = psA.tile([128, 512], f32, tag=f"y{h}")
                    nc.tensor.matmul(yA[:], W(2 * l), xa_mv[h],
                                     start=True, stop=True)
                    if TR_FROM_PSUM:
                        # StreamTranspose needs same src/dst dtype: f32r in
                        # (bitcast of the f32 PSUM), f32r-rounded out
                        xB = cv.tile([128, 512], f32, tag=f"xb{h}")
                        nc.vector.transpose(xB[:], yA[:])
                        xB_mv = xB[:]
                    else:
                        yc = cv.tile([128, 512], bf16, tag=f"yc{h}")
                        nc.scalar.activation(yc[:], yA[:], AF.Copy)
                        xBb = cv.tile([128, 512], bf16, tag=f"xb{h}")
                        nc.vector.transpose(xBb[:].bitcast(u32),
                                            yc[:].bitcast(u32))
                        xB_mv = xBb[:]
                    zB = psB.tile([128, 512], f32, tag=f"z{h}")
                    nc.tensor.matmul(zB[:], W(2 * l + 1), xB_mv,
                                     start=True, stop=True)
                    if l < N_LAYERS - 1:
                        if TR_FROM_PSUM:
                            xt = xp.tile([128, 512], f32, tag=f"x{h}")
                            nc.vector.transpose(xt[:], zB[:])
                            xa_mv[h] = xt[:]
                        else:
                            zc = cv.tile([128, 512], bf16, tag=f"zc{h}")
                            nc.scalar.activation(zc[:], zB[:], AF.Copy)
                            xt = xp.tile([128, 512], bf16, tag=f"x{h}")
                            nc.vector.transpose(xt[:].bitcast(u32),
                                                zc[:].bitcast(u32))
                            xa_mv[h] = xt[:]
                    else:
                        zb[h] = zB
                        meas_front(h, zB)

            # ---- measurement (fronts already emitted at l == 3)
            # sq stored as (h, u3, b0, j5); sqB as (j5, b0, h, u3)
            o1B = psO.tile([32, 1024], f32, tag="o1B")
            for g in range(2):
                nc.tensor.matmul(o1B[:, 512 * g:512 * g + 512], w1_t,
                                 sqB[:, 512 * g:512 * g + 512],
                                 start=True, stop=True)

            # o1t[j5, (h, u, (b0,o))] and o1u[(h,u), (j5, (b0,o))]
            o1t = sm.tile([32, 1024], f32, tag="o1t")
            for h in range(2):
                nc.vector.transpose(o1t[:, 512 * h:512 * h + 512],
                                    o1A[h][:])
            # TR2 split at the j5 midpoint: each half transposes as soon
            # as its o1B matmul lands instead of waiting for both
            o1u = sm.tile([32, 1024], f32, tag="o1u")
            for g in range(2):
                nc.vector.transpose(o1u[:, 512 * g:512 * g + 512],
                                    o1B[:, 512 * g:512 * g + 512])

            # q0..4: stationary-data matmul -> [64, 5] PSUM. Stationary
            # free emitted in natural (g, b0, b1) order (collapses to one
            # AP dim); rows land as r = (g, b0, b1) -- the host gather
            # un-swaps b0/b1 (ROWPERM)
            sa = o1t[:].rearrange("p (gb o) -> p gb o", o=16)[:, :, 0]
            outa = psS.tile([64, 5], f32, tag="vp1")
            nc.tensor.matmul(outa[:], sa, ca_t, start=True, stop=True)
            outa_sb = sm.tile([64, 5], f32, tag="outa")
            nc.vector.tensor_copy(outa_sb[:], outa[:])

            # q5..9: two partial strided reduces (one per o1u half, each
            # ready right after its transpose) + a tiny add
            out59p = sm.tile([32, 20], f32, tag="out59p")
            for g in range(2):
                nc.vector.tensor_reduce(
                    out59p[:, 10 * g:10 * g + 10]
                        .rearrange("p (b o) -> p b o", b=2),
                    o1u[:, 512 * g:512 * g + 512]
                        .rearrange("p (j b o) -> p b o j",
                                   j=16, b=2, o=16)[:, :, 1:6, :],
                    mybir.AxisListType.X, OP.add)
            out59 = sm.tile([32, 10], f32, tag="out59")
            nc.vector.tensor_tensor(out59[:], out59p[:, 0:10],
                                    out59p[:, 10:20], OP.add)

            # second GPSIMD touch near the tail so the postamble's
            # semaphore range-clear does not pay a wake-up
            nc.gpsimd.tensor_copy(gdum[:], sq[0:1, 0:1])
            # q0..4: psum partitions already in sample-row order
            nc.sync.dma_start(out_d[:, 0:5], outa_sb[:])
            # q5..9: partition p = (b0, (h,u3)); write rows in the same
            # swapped convention r = 4k + 2*b0 + b1 the host un-permutes
            o59v = out59[:].rearrange("p (b1 o) -> p b1 o", b1=2)
            outv = out_d.rearrange("(k b0 b1) q -> k b0 b1 q", b0=2, b1=2)
            for b0 in range(2):
                eng = nc.sync if b0 == 0 else nc.scalar
                eng.dma_start(outv[:, b0, :, 5:10],
                              o59v[16 * b0:16 * b0 + 16])

    nc.finalize()
    return nc


def _get_module():
    if "nc" not in _BUILD_CACHE:
        _BUILD_CACHE["nc"] = _build_module()
    return _BUILD_CACHE["nc"]


# ---------------------------------------------------------------- entrypoint
def kernel(inputs, theta):
    inputs = np.asarray(inputs, dtype=np.float32)
    theta = np.asarray(theta, dtype=np.float32)
    assert inputs.shape == (B_TOTAL, N_QUBITS)

    from concourse.bass_utils import run_bass_kernel_spmd

    nc = _get_module()
    wpack = _pack_w(theta)
    in_maps = []
    for c in range(N_CORES):
        shard = np.ascontiguousarray(inputs[B_CORE * c:B_CORE * (c + 1)])
        in_maps.append({"xin": shard, "wpack": wpack})
    res = run_bass_kernel_spmd(nc, in_maps, core_ids=list(range(N_CORES)))
    out = np.empty((B_TOTAL, N_QUBITS), np.float32)
    for c in range(N_CORES):
        out[B_CORE * c + ROWPERM] = res.results[c]["out"]
    return out
